# revision 44
# baseline (speedup 1.0000x reference)
"""Trainium2 Bass kernel for nn_BiLSTMWithLM (B=64, T=1024, D_IN=400).

Data-parallel over batch: 8 cores x 8 sequences each.
  S0/S1: bidirectional LSTM scans (layer 0 then layer 1). fwd and bwd run
      as two independent dependency chains whose per-step latencies overlap.
      Input projections are produced just-in-time inside the scan (JIT
      producer matmul groups + psum drains spread across step slots) —
      there is no separate projection phase and no proj DRAM round trip.
      Per step: identity-matmul preloads PSUM with the input projection,
      recurrent matmuls accumulate against dual stationaries
      [2*Whh^T | -Whh^T] taking t2 = sig(o)*sig(2c) and sig(o) as moving
      operands (h = 2*t2 - sig(o) stays OFF the critical cycle; h for the
      next layer is materialized once per chunk in bulk). Gate order
      [i,f,o,g] with g-rows pre-scaled 2x on host: ONE sigmoid covers all
      four gate slabs (tanh(g) = 2*sig(2g) - 1), and tanh(c) = 2*sig(2c)-1
      via the activation input scale. Cell update: A/2 = (sig(2g)-0.5)*
      sig(i) in one DVE op, then c = 2*(A/2) + f*c_prev; the three DVE ops
      run per-chain back-to-back (an on-cycle Pool op costs ~260ns vs
      ~155ns on DVE due to dispatch + Q7 launch).
  P3: head. BN1/linear/BN2 folded on host into LW/LB; computes
      u = tanh(LW @ l1out + LB) and the logit-difference drive
      du = w3s . u + K0, scattered to dud[blk, b, w].
  P4: context scan reformulated as a scalar recurrence on the logit diff
      d_t = du_t + g*d_{t-1} - dl*sp(d_{t-1}) + a*d_{t-2} - b*sp(d_{t-2}),
      solved by Jacobi fixed-point iteration (contraction ~0.085/iter)
      vectorized over [b*16+blk, w] partitions with a 20-col redundant
      halo (influence travels 2 cols/iter so block seams never reach the
      final region); lo0 = -softplus(d), lo1 = d - softplus(d).
"""
import os
import sys

sys.path.insert(0, "/opt/trn_rl_repo")

import numpy as np
import ml_dtypes

import concourse.bass as bass
import concourse.bacc as bacc
import concourse.mybir as mybir
from concourse import tile
from concourse.bass_utils import run_bass_kernel_spmd
from concourse.kernels.tile_matmul import matmul_tile_kernel
from contextlib import ExitStack

BF16 = mybir.dt.bfloat16
F32 = mybir.dt.float32
AF = mybir.ActivationFunctionType
OP = mybir.AluOpType

B, D_IN, H = 64, 400, 128
T = int(os.environ.get("KERNEL_T", "1024"))
N_CORES = 8
BL = B // N_CORES          # 8 local sequences
N = T * BL                 # columns, n = t*8 + b
C = 64                     # scan chunk (steps per DMA chunk)
N_JACOBI = 5
EPS = 1e-5


def _bf16(x):
    return np.asarray(x, dtype=ml_dtypes.bfloat16)


def _perm_gates(w):
    # torch order [i,f,g,o] -> [i,f,o,g], with g rows scaled 2x so the
    # kernel can use tanh(g) = 2*sigmoid(2g) - 1 with a single sigmoid.
    i, f, g, o = np.split(np.asarray(w), 4, axis=0)
    return np.concatenate([i, f, o, 2.0 * g], axis=0)


_BUILD_CACHE = {}


def _build():
    if T in _BUILD_CACHE:
        return _BUILD_CACHE[T]

    nc = bacc.Bacc("TRN2", target_bir_lowering=False, debug=False,
                   num_devices=N_CORES)

    def din(name, shape, dtype):
        return nc.dram_tensor(name, shape, dtype, kind="ExternalInput").ap()

    def dscratch(name, shape, dtype):
        return nc.dram_tensor(name, shape, dtype).ap()

    # inputs
    xk = din("xk", [128, 4, N], BF16)               # aug x, kxn for P1
    w0 = {d: din(f"w0{d}", [128, 4, 512], BF16) for d in "fb"}
    w1 = {d: din(f"w1{d}", [128, 3, 512], BF16) for d in "fb"}
    whh0 = {d: din(f"whh0{d}", [128, 1024], BF16) for d in "fb"}
    whh1 = {d: din(f"whh1{d}", [128, 1024], BF16) for d in "fb"}
    lwk = din("lwk", [128, 2, 64], BF16)            # LW.T tiled
    lbv = din("lbv", [64, 1], F32)                  # LB bias
    w3s = din("w3s", [64, 1], BF16)                 # head diff vector
    coef = din("coef", [128, 8], F32)               # [g, -dl, a, -b, K0]
    cmsk = din("cmsk", [128, 20], F32)              # 0 at seq-start rows
    ident = din("ident", [128, 128], BF16)
    outv = nc.dram_tensor("outv", [N, 2], F32, kind="ExternalOutput").ap()

    # scratch
    l0out = dscratch("l0out", [128, 3, N], BF16)
    l1out = dscratch("l1out", [128, 2, N], BF16)
    # du as [blk, b, w]: time-block-major so the P3 scatter is a plain
    # first-axis index (clean DMA region for dependency tracking)
    W4 = T // 16
    dud = dscratch("dud", [16, BL, W4], F32)

    with tile.TileContext(nc) as tc:
        # ---- init: l0out kb=2 block (ones row at p=0, zeros elsewhere) ----
        with ExitStack() as ctx:
            pool = ctx.enter_context(tc.tile_pool(name="initp", bufs=1))
            ozt = pool.tile([128, 512], BF16)
            nc.vector.memset(ozt[:], 0.0)
            nc.vector.memset(ozt[0:1, :], 1.0)
            for i in range(N // 512):
                nc.sync.dma_start(l0out[:, 2, bass.ts(i, 512)], ozt[:])

        # ---- scan helper ----
        # fwd and bwd run as two independent dependency chains (separate
        # psum/ACT/DVE/Pool instructions) so their per-step latencies
        # overlap. Gate order [i,f,o,g] with g-rows pre-scaled 2x on host:
        # one sigmoid covers all four slabs (tanh(g) = 2*sig(2g) - 1), and
        # tanh(c) = 2*sig(2c) - 1 via the activation input scale. Cell
        # update runs partly on the (otherwise idle) Pool engine.
        # h itself is OFF the critical cycle: the recurrent matmul takes
        # t2 = sig(o)*sig(2c) and sig(o) as moving operands against dual
        # stationaries [2*Whh^T | -Whh^T] (h = 2*t2 - sig(o)); the h tensor
        # for the next layer is materialized once per chunk in bulk.
        def scan(layer, mov, wst_d, kb_in, whhf_d, whhb_d, out_ap, kb_f, kb_b,
                 head=False):
            with ExitStack() as ctx:
                cpool = ctx.enter_context(tc.tile_pool(name=f"wh{layer}", bufs=1))
                whf = cpool.tile([128, 1024], BF16)
                whb = cpool.tile([128, 1024], BF16)
                idt = cpool.tile([128, 128], BF16)
                nc.sync.dma_start(whf[:], whhf_d[:])
                nc.sync.dma_start(whb[:], whhb_d[:])
                nc.sync.dma_start(idt[:], ident[:])
                wst = {}
                for d in "fb":
                    wst[d] = cpool.tile([128, kb_in, 512], BF16,
                                        name=f"wst{d}")
                    nc.sync.dma_start(wst[d][:], wst_d[d][:])
                if head:
                    lw_sb = cpool.tile([128, 2, 64], BF16)
                    lb_sb = cpool.tile([64, 1], F32)
                    w3_sb = cpool.tile([64, 1], BF16)
                    nc.sync.dma_start(lw_sb[:], lwk[:])
                    nc.sync.dma_start(lb_sb[:], lbv[:])
                    nc.sync.dma_start(w3_sb[:], w3s[:])

                ppool = ctx.enter_context(tc.tile_pool(name=f"pj{layer}", bufs=2))
                mvp = ctx.enter_context(tc.tile_pool(name=f"mv{layer}", bufs=2))
                prps = ctx.enter_context(
                    tc.tile_pool(name=f"prps{layer}", bufs=2, space="PSUM"))

                # JIT projection producer: computes the input projections for
                # fwd chunk ch / bwd chunk (n-1-ch) into SBUF while the scan
                # (latency-bound, PE ~idle) runs. Drains are emitted by the
                # caller spread across step slots to avoid blocking the
                # in-order DVE with big copies.
                def produce_loads(ch):
                    t0p, tb0p = ch * C, T - C - ch * C
                    mf = mvp.tile([128, kb_in, C * BL], BF16, tag="mf",
                                  name="mf")
                    mb = mvp.tile([128, kb_in, C * BL], BF16, tag="mb",
                                  name="mb")
                    nc.sync.dma_start(mf[:], mov[:, :, t0p * BL:(t0p + C) * BL])
                    nc.sync.dma_start(mb[:], mov[:, :, tb0p * BL:(tb0p + C) * BL])
                    pf = ppool.tile([128, 4, C * BL], BF16, tag="pf", name="pf")
                    pb = ppool.tile([128, 4, C * BL], BF16, tag="pb", name="pb")
                    return [pf, pb, mf, mb]

                def produce_group(pstate, k):
                    # one gate-block matmul group + its psum drain (~1.3us);
                    # emitted between step_pairs so it fills the cycle's idle
                    # PE/DVE windows instead of stalling the in-order queues
                    pf, pb, mf, mb = pstate
                    d, nt = ("f", k) if k < 4 else ("b", k - 4)
                    msb, dst = (mf, pf) if k < 4 else (mb, pb)
                    pp = prps.tile([128, C * BL], F32, tag="prp", name="prp")
                    for kb in range(kb_in):
                        nc.tensor.matmul(
                            pp[:], wst[d][:, kb, nt * 128:(nt + 1) * 128],
                            msb[:, kb, :], start=(kb == 0),
                            stop=(kb == kb_in - 1),
                            skip_group_check=True)
                    nc.vector.tensor_copy(dst[:, nt, :], pp[:])
                # head mode retains ALL chunk h-tiles so the output head can
                # consume time-chunk (ch, n-1-ch) pairs inside the loop
                hpool = ctx.enter_context(tc.tile_pool(
                    name=f"hc{layer}", bufs=(T // C) if head else 2))
                schp = {d: ctx.enter_context(
                    tc.tile_pool(name=f"sc{layer}{d}", bufs=2)) for d in "fb"}
                spool = {d: ctx.enter_context(
                    tc.tile_pool(name=f"s{layer}{d}", bufs=3)) for d in "fb"}
                cstp = {d: ctx.enter_context(
                    tc.tile_pool(name=f"cst{layer}{d}", bufs=2)) for d in "fb"}
                psum = {d: ctx.enter_context(
                    tc.tile_pool(name=f"ps{layer}{d}", bufs=3, space="PSUM"))
                    for d in "fb"}

                # per-dir state: (t2_prev_slice, o_prev_slice, cprev)
                st = {"f": (None, None, None), "b": (None, None, None)}
                wh = {"f": whf, "b": whb}

                # Stage-interleaved emission: each engine's instruction
                # stream alternates f/b so the in-order engines never queue
                # a dependent op of one chain ahead of the other chain's
                # ready op (which would serialize the chains).
                def step_pair(step, slots):
                    ps, S, t1, A, c1, cnew, s2c = ({} for _ in range(7))
                    for d, (proj_sl, Sch, t2ch, sl) in slots.items():
                        t2p, op_, _ = st[d]
                        ps[d] = psum[d].tile([128, 32], F32, name=f"ps{d}")
                        nc.tensor.matmul(ps[d][:], idt[:], proj_sl,
                                         start=True, stop=(step == 0),
                                         skip_group_check=True)
                        if step > 0:
                            for g in range(4):
                                nc.tensor.matmul(
                                    ps[d][:, g * 8:(g + 1) * 8],
                                    wh[d][:, g * 128:(g + 1) * 128], t2p,
                                    start=False, stop=False,
                                    skip_group_check=True)
                            for g in range(4):
                                nc.tensor.matmul(
                                    ps[d][:, g * 8:(g + 1) * 8],
                                    wh[d][:, 512 + g * 128:512 + (g + 1) * 128],
                                    op_, start=False, stop=(g == 3),
                                    skip_group_check=True)
                    for d, (proj_sl, Sch, t2ch, sl) in slots.items():
                        S[d] = Sch[:, sl, :]
                        nc.scalar.activation(S[d], ps[d][:], AF.Sigmoid)
                    # Cell-update trios run per-chain back-to-back on DVE
                    # (f's three ops, then b's): an on-cycle Pool op costs
                    # ~260ns (dispatch + Q7 launch) vs ~155ns on DVE, and
                    # per-chain grouping keeps the in-order DVE from gating
                    # one chain on the other's sigmoid.
                    for d in slots:
                        _, _, cprev = st[d]
                        # A/2 = (sig(2g) - 0.5) * sig(i)  [= sig(i)*tanh(g)/2]
                        t1[d] = spool[d].tile([128, 8], F32, tag="t1", name=f"t1{d}")
                        nc.vector.scalar_tensor_tensor(
                            t1[d][:], S[d][:, 24:32], -0.5, S[d][:, 0:8],
                            OP.add, OP.mult)
                        cnew[d] = cstp[d].tile([128, 8], F32, tag="c", name=f"c{d}")
                        if step > 0:
                            c1[d] = cstp[d].tile([128, 8], F32, tag="c1", name=f"c1{d}")
                            nc.vector.tensor_tensor(c1[d][:], S[d][:, 8:16],
                                                    cprev[:], OP.mult)
                            nc.vector.scalar_tensor_tensor(
                                cnew[d][:], t1[d][:], 2.0, c1[d][:],
                                OP.mult, OP.add)
                        else:
                            nc.vector.tensor_scalar(
                                cnew[d][:], t1[d][:], 2.0, None, OP.mult)
                    for d in slots:
                        s2c[d] = spool[d].tile([128, 8], F32, tag="s2c", name=f"s2c{d}")
                        nc.scalar.activation(s2c[d][:], cnew[d][:],
                                             AF.Sigmoid, scale=2.0)
                    for d, (proj_sl, Sch, t2ch, sl) in slots.items():
                        t2sl = t2ch[:, sl, :]
                        nc.vector.tensor_tensor(t2sl, S[d][:, 16:24],
                                                s2c[d][:], OP.mult)
                        st[d] = (t2sl, S[d][:, 16:24], cnew[d])

                # head: u = tanh(LW @ [h_f; h_b] + LB), du = w3s . u,
                # scattered to dud[tc]; consumes retained SBUF h-tiles.
                ht_f, ht_b = {}, {}

                def head_consume(tc_):
                    pu = prps.tile([128, C * BL], F32, tag="prp", name="hpu")
                    nc.tensor.matmul(pu[0:64, :], lw_sb[:, 0, :],
                                     ht_f[tc_][:], start=True, stop=False,
                                     skip_group_check=True)
                    nc.tensor.matmul(pu[0:64, :], lw_sb[:, 1, :],
                                     ht_b[tc_][:], start=False, stop=True,
                                     skip_group_check=True)
                    ut = spool["f"].tile([64, C * BL], BF16, tag="hut",
                                         name="hut")
                    nc.scalar.activation(ut[:], pu[0:64, :], AF.Tanh,
                                         bias=lb_sb[:])
                    pd = prps.tile([128, C * BL], F32, tag="prp", name="hpd")
                    nc.tensor.matmul(pd[0:1, :], w3_sb[:], ut[:],
                                     start=True, stop=True,
                                     skip_group_check=True)
                    dt_ = spool["f"].tile([1, C * BL], F32, tag="hdt",
                                          name="hdt")
                    nc.vector.tensor_copy(dt_[:], pd[0:1, :])
                    src = dt_[:].rearrange("o (t b) -> o t b", b=BL)
                    dst = dud[tc_:tc_ + 1].rearrange("o b t -> o t b")
                    nc.sync.dma_start(dst, src)

                n_chunks = T // C
                cur = produce_loads(0)
                for k in range(8):
                    produce_group(cur, k)
                pf, pb = cur[0], cur[1]
                for ch in range(n_chunks):
                    t0 = ch * C
                    tb0 = T - C - t0  # bwd chunk start (ascending t)
                    nxt = produce_loads(ch + 1) if ch + 1 < n_chunks else None
                    Sf_ch = schp["f"].tile([128, C, 32], BF16, tag="Sch")
                    Sb_ch = schp["b"].tile([128, C, 32], BF16, tag="Sch")
                    t2f_ch = schp["f"].tile([128, C, 8], BF16, tag="t2ch")
                    t2b_ch = schp["b"].tile([128, C, 8], BF16, tag="t2ch")
                    hf_ch = hpool.tile([128, C * BL], BF16, tag="hf")
                    hb_ch = hpool.tile([128, C * BL], BF16, tag="hb")

                    for c in range(C):
                        step = t0 + c
                        cb = C - 1 - c  # bwd slot (reversed within chunk)
                        step_pair(step, {
                            "f": (pf[:, :, c * BL:(c + 1) * BL], Sf_ch, t2f_ch, c),
                            "b": (pb[:, :, cb * BL:(cb + 1) * BL], Sb_ch, t2b_ch, cb),
                        })
                        if nxt is not None and c % 8 == 4:
                            produce_group(nxt, c // 8)

                    # bulk h = 2*t2 - sig(o) for the whole chunk (off-cycle)
                    hf_v = hf_ch[:].rearrange("p (c x) -> p c x", x=BL)
                    hb_v = hb_ch[:].rearrange("p (c x) -> p c x", x=BL)
                    nc.vector.scalar_tensor_tensor(
                        hf_v, t2f_ch[:], 2.0, Sf_ch[:, :, 16:24],
                        OP.mult, OP.subtract)
                    nc.vector.scalar_tensor_tensor(
                        hb_v, t2b_ch[:], 2.0, Sb_ch[:, :, 16:24],
                        OP.mult, OP.subtract)
                    nc.sync.dma_start(
                        out_ap[:, kb_f, t0 * BL:(t0 + C) * BL], hf_ch[:])
                    nc.sync.dma_start(
                        out_ap[:, kb_b, tb0 * BL:(tb0 + C) * BL], hb_ch[:])
                    if head:
                        ht_f[ch] = hf_ch
                        ht_b[n_chunks - 1 - ch] = hb_ch
                        if ch >= n_chunks // 2:
                            # time-chunks ch and n-1-ch just became complete
                            head_consume(ch)
                            head_consume(n_chunks - 1 - ch)
                    if nxt is not None:
                        pf, pb = nxt[0], nxt[1]

        # ---- S0 (with JIT layer-0 input projections) ----
        scan(0, xk, w0, 4, whh0["f"], whh0["b"], l0out, 0, 1)

        # ---- S1 (with JIT layer-1 input projections) ----
        scan(1, l0out, w1, 3, whh1["f"], whh1["b"], l1out, 0, 1,
             head=True)

        # ---- P4: context solve (jacobi) + output ----
        # Packed [b*16 + blk, w] across all 128 partitions, with a HALO-col
        # redundant left margin per block: influence travels 2 cols/iter, so
        # HALO=20 > 2*N_JACOBI keeps block seams out of the final region.
        # Seq-start rows (p % 16 == 0) mask their halo (and its softplus) to
        # zero, reproducing the d=0, sp=0 start condition.
        HALO, EXT = 20, 20 + W4
        with ExitStack() as ctx:
            cpool = ctx.enter_context(tc.tile_pool(name="ctxc", bufs=1))
            cf = cpool.tile([128, 8], F32)
            nc.sync.dma_start(cf[:], coef[:])
            mskt = cpool.tile([128, HALO], F32)
            nc.sync.dma_start(mskt[:], cmsk[:])
            d0x = cpool.tile([128, EXT], F32)
            nc.vector.memset(d0x[:, 0:HALO], 0.0)
            for b in range(BL):
                # partition b*16+k holds block k of sequence b
                nc.sync.dma_start(d0x[b * 16:(b + 1) * 16, HALO:EXT],
                                  dud[:, b, :])
                # halo: last HALO cols of block k-1 (rows b*16+1..b*16+15;
                # seq-start rows p=b*16 keep the memset zeros)
                nc.sync.dma_start(d0x[b * 16 + 1:(b + 1) * 16, 0:HALO],
                                  dud[0:15, b, W4 - HALO:W4])
            # += K0 everywhere, then re-zero seq-start halos (K0 was added)
            nc.vector.tensor_scalar(d0x[:], d0x[:], cf[:, 4:5], None, OP.add)
            nc.vector.tensor_tensor(d0x[:, 0:HALO], d0x[:, 0:HALO], mskt[:],
                                    OP.mult)
            jp = ctx.enter_context(tc.tile_pool(name="jac", bufs=2))
            sp_p = ctx.enter_context(tc.tile_pool(name="jsp", bufs=2))
            d_cur = d0x
            g_, dl_, a_, b_ = (cf[:, 0:1], cf[:, 1:2], cf[:, 2:3], cf[:, 3:4])

            def stt(out, in0, scal, in1):
                nc.vector.scalar_tensor_tensor(out, in0, scal, in1,
                                               OP.mult, OP.add)

            def softplus(out_ap, in_ap):
                # Softplus has no ACT table on this build: ln(1 + exp(x)),
                # with the +1 folded into Ln's constant bias (no DVE hop).
                # d stays small (|d| < ~3) so no overflow concerns.
                nc.scalar.activation(out_ap, in_ap, AF.Exp)
                nc.scalar.activation(out_ap, out_ap, AF.Ln, bias=1.0)

            for it in range(N_JACOBI):
                sp = sp_p.tile([128, EXT], F32, tag="sp")
                softplus(sp[:], d_cur[:])
                nc.vector.tensor_tensor(sp[:, 0:HALO], sp[:, 0:HALO],
                                        mskt[:], OP.mult)
                acc = jp.tile([128, EXT], F32, tag="acc")
                nc.vector.tensor_copy(acc[:, 0:2], d0x[:, 0:2])
                stt(acc[:, 1:EXT], d_cur[:, 0:EXT - 1], g_, d0x[:, 1:EXT])
                stt(acc[:, 1:EXT], sp[:, 0:EXT - 1], dl_, acc[:, 1:EXT])
                stt(acc[:, 2:EXT], d_cur[:, 0:EXT - 2], a_, acc[:, 2:EXT])
                stt(acc[:, 2:EXT], sp[:, 0:EXT - 2], b_, acc[:, 2:EXT])
                d_cur = acc

            spf = sp_p.tile([128, EXT], F32, tag="sp")
            softplus(spf[:], d_cur[:])
            lo = cpool.tile([128, W4, 2], F32)
            nc.vector.tensor_scalar(lo[:, :, 0], spf[:, HALO:EXT], -1.0,
                                    None, OP.mult)
            nc.vector.tensor_tensor(lo[:, :, 1], d_cur[:, HALO:EXT],
                                    spf[:, HALO:EXT], OP.subtract)
            out_view = outv.rearrange("(b k w) x -> (b k) w x", b=BL, k=16)
            nc.sync.dma_start(out_view, lo[:])

    nc.compile()
    _BUILD_CACHE[T] = nc
    return nc


# ---------------------------------------------------------------------------
# host-side prep + execution
# ---------------------------------------------------------------------------
def _prep_shared(inputs):
    sh = {}
    for l, (din_, kpad, wkey) in enumerate(((D_IN, 512, "w0"),
                                            (256, 384, "w1"))):
        for d, suf in (("f", ""), ("b", "r")):
            wih = _perm_gates(inputs[f"w_ih_l{l}{suf}"])       # [512, din]
            whh = _perm_gates(inputs[f"w_hh_l{l}{suf}"])       # [512, 128]
            bias = _perm_gates(
                np.asarray(inputs[f"b_ih_l{l}{suf}"])
                + np.asarray(inputs[f"b_hh_l{l}{suf}"]))       # [512]
            aug = np.zeros((kpad, 512), np.float32)
            aug[:din_] = np.asarray(wih, np.float32).T
            aug[din_] = bias
            sh[f"{wkey}{d}"] = _bf16(
                aug.reshape(kpad // 128, 128, 512).transpose(1, 0, 2))
            wT = np.asarray(whh, np.float32).T          # [128, 512]
            sh[f"whh{l}{d}"] = _bf16(
                np.concatenate([2.0 * wT, -wT], axis=1))  # h = 2*t2 - sig(o)

    g1, b1 = np.asarray(inputs["bn1_g"]), np.asarray(inputs["bn1_b"])
    m1, v1 = np.asarray(inputs["bn1_m"]), np.asarray(inputs["bn1_v"])
    s1 = g1 / np.sqrt(v1 + EPS)
    t1 = b1 - m1 * s1
    lin_w = np.asarray(inputs["lin_w"])
    LW = lin_w * s1[None, :]
    LB = np.asarray(inputs["lin_b"]) + lin_w @ t1
    g2, b2 = np.asarray(inputs["bn2_g"]), np.asarray(inputs["bn2_b"])
    m2, v2 = np.asarray(inputs["bn2_m"]), np.asarray(inputs["bn2_v"])
    s2 = g2 / np.sqrt(v2 + EPS)
    t2 = b2 - m2 * s2
    out_w, out_b = np.asarray(inputs["out_w"]), np.asarray(inputs["out_b"])
    W1, W2, W3 = out_w[:, 0:2], out_w[:, 2:4], out_w[:, 4:68]
    w3d = W3[1] - W3[0]
    K0 = (out_b[1] - out_b[0]) + t2 @ w3d
    w1d, w2d = W1[1] - W1[0], W2[1] - W2[0]
    alpha, beta = w1d[1], w1d[0] + w1d[1]
    gamma, delta = w2d[1], w2d[0] + w2d[1]

    sh["lwk"] = _bf16(LW.T.reshape(2, 128, 64).transpose(1, 0, 2))
    sh["lbv"] = np.asarray(LB, np.float32).reshape(64, 1)
    sh["w3s"] = _bf16((w3d * s2).reshape(64, 1))
    coefs = np.zeros((128, 8), np.float32)
    coefs[:, 0] = gamma
    coefs[:, 1] = -delta
    coefs[:, 2] = alpha
    coefs[:, 3] = -beta
    coefs[:, 4] = K0
    sh["coef"] = coefs
    cmsk = np.ones((128, 20), np.float32)
    cmsk[0::16, :] = 0.0          # sequence-start rows: zero halo
    sh["cmsk"] = cmsk
    sh["ident"] = _bf16(np.eye(128, dtype=np.float32))
    return sh


def _prep_core(x_core):
    # x_core: [BL, T, 400] -> aug kxn [128, 4, T*BL] bf16
    xt = np.zeros((512, T * BL), np.float32)
    xt[:D_IN] = np.asarray(x_core, np.float32).transpose(2, 1, 0).reshape(
        D_IN, T * BL)
    xt[D_IN] = 1.0
    return _bf16(xt.reshape(4, 128, T * BL).transpose(1, 0, 2))


def kernel(**inputs):
    nc = _build()
    sh = _prep_shared(inputs)
    x = np.asarray(inputs["x"], np.float32)
    in_maps = []
    for cidx in range(N_CORES):
        m = dict(sh)
        m["xk"] = _prep_core(x[cidx * BL:(cidx + 1) * BL])
        in_maps.append(m)
    res = run_bass_kernel_spmd(nc, in_maps, list(range(N_CORES)))
    outs = [np.asarray(res.results[i]["outv"], np.float32)
            for i in range(N_CORES)]
    return np.concatenate(outs, axis=0)


if __name__ == "__main__":
    import time
    t0 = time.time()
    print(f"building T={T}...")
    _build()
    print(f"built in {time.time() - t0:.1f}s")



# revision 46
# speedup vs baseline: 1.1089x; 1.1089x over previous
"""Trainium2 Bass kernel for nn_BiLSTMWithLM (B=64, T=1024, D_IN=400).

Data-parallel over batch: 8 cores x 8 sequences each.
  S0/S1: bidirectional LSTM scans (layer 0 then layer 1). fwd and bwd run
      as two independent dependency chains whose per-step latencies overlap.
      Input projections are produced just-in-time inside the scan (JIT
      producer matmul groups + psum drains spread across step slots) —
      there is no separate projection phase and no proj DRAM round trip.
      Per step: identity-matmul preloads PSUM with the input projection,
      recurrent matmuls accumulate against dual stationaries
      [2*Whh^T | -Whh^T] taking t2 = sig(o)*sig(2c) and sig(o) as moving
      operands (h = 2*t2 - sig(o) stays OFF the critical cycle; h for the
      next layer is materialized once per chunk in bulk). Gate order
      [i,f,o,g] with g-rows pre-scaled 2x on host: ONE sigmoid covers all
      four gate slabs (tanh(g) = 2*sig(2g) - 1), and tanh(c) = 2*sig(2c)-1
      via the activation input scale. Cell update: A/2 = (sig(2g)-0.5)*
      sig(i) in one DVE op, then c = 2*(A/2) + f*c_prev; the three DVE ops
      run per-chain back-to-back (an on-cycle Pool op costs ~260ns vs
      ~155ns on DVE due to dispatch + Q7 launch).
  head (inside S1): BN1/linear/BN2 folded on host into LW/LB; u =
      tanh(LW @ [h_f; h_b] + LB) and the logit-difference drive
      du = w3s . u are computed per time-chunk from SBUF-retained h tiles
      during S1 iterations ch >= 8 (when both directions of time-chunks
      ch and 15-ch are complete), scattered to dud[blk, b, w].
  P4: context scan reformulated as a scalar recurrence on the logit diff
      d_t = du_t + g*d_{t-1} - dl*sp(d_{t-1}) + a*d_{t-2} - b*sp(d_{t-2}),
      solved by Jacobi fixed-point iteration (contraction ~0.085/iter)
      vectorized over [b*16+blk, w] partitions with a 20-col redundant
      halo (influence travels 2 cols/iter so block seams never reach the
      final region); lo0 = -softplus(d), lo1 = d - softplus(d).
"""
import os
import sys

sys.path.insert(0, "/opt/trn_rl_repo")

import numpy as np
import ml_dtypes

import concourse.bass as bass
import concourse.bacc as bacc
import concourse.mybir as mybir
from concourse import tile
from concourse.bass_utils import run_bass_kernel_spmd
from concourse.kernels.tile_matmul import matmul_tile_kernel
from contextlib import ExitStack

BF16 = mybir.dt.bfloat16
F32 = mybir.dt.float32
AF = mybir.ActivationFunctionType
OP = mybir.AluOpType

B, D_IN, H = 64, 400, 128
T = int(os.environ.get("KERNEL_T", "1024"))
N_CORES = 8
BL = B // N_CORES          # 8 local sequences
N = T * BL                 # columns, n = t*8 + b
C = 64                     # scan chunk (steps per DMA chunk)
N_JACOBI = 5
EPS = 1e-5


def _bf16(x):
    return np.asarray(x, dtype=ml_dtypes.bfloat16)


def _perm_gates(w):
    # torch order [i,f,g,o] -> [i,f,o,g], with g rows scaled 2x so the
    # kernel can use tanh(g) = 2*sigmoid(2g) - 1 with a single sigmoid.
    i, f, g, o = np.split(np.asarray(w), 4, axis=0)
    return np.concatenate([i, f, o, 2.0 * g], axis=0)


_BUILD_CACHE = {}


def _build():
    if T in _BUILD_CACHE:
        return _BUILD_CACHE[T]

    nc = bacc.Bacc("TRN2", target_bir_lowering=False, debug=False,
                   num_devices=N_CORES)

    def din(name, shape, dtype):
        return nc.dram_tensor(name, shape, dtype, kind="ExternalInput").ap()

    def dscratch(name, shape, dtype):
        return nc.dram_tensor(name, shape, dtype).ap()

    # inputs
    xk = din("xk", [128, 4, N], BF16)               # aug x, kxn for P1
    w0 = {d: din(f"w0{d}", [128, 4, 512], BF16) for d in "fb"}
    w1 = {d: din(f"w1{d}", [128, 3, 512], BF16) for d in "fb"}
    whh0 = {d: din(f"whh0{d}", [128, 1024], BF16) for d in "fb"}
    whh1 = {d: din(f"whh1{d}", [128, 1024], BF16) for d in "fb"}
    lwk = din("lwk", [128, 2, 64], BF16)            # LW.T tiled
    lbv = din("lbv", [64, 1], F32)                  # LB bias
    w3s = din("w3s", [64, 1], BF16)                 # head diff vector
    coef = din("coef", [128, 8], F32)               # [g, -dl, a, -b, K0]
    cmsk = din("cmsk", [128, 20], F32)              # 0 at seq-start rows
    ident = din("ident", [128, 128], BF16)
    outv = nc.dram_tensor("outv", [N, 2], F32, kind="ExternalOutput").ap()

    # scratch
    l0out = dscratch("l0out", [128, 3, N], BF16)
    # du as [blk, b, w]: time-block-major so the P3 scatter is a plain
    # first-axis index (clean DMA region for dependency tracking)
    W4 = T // 16
    dud = dscratch("dud", [16, BL, W4], F32)

    with tile.TileContext(nc) as tc:
        # ---- init: l0out kb=2 block (ones row at p=0, zeros elsewhere) ----
        with ExitStack() as ctx:
            pool = ctx.enter_context(tc.tile_pool(name="initp", bufs=1))
            ozt = pool.tile([128, 512], BF16)
            nc.vector.memset(ozt[:], 0.0)
            nc.vector.memset(ozt[0:1, :], 1.0)
            for i in range(N // 512):
                nc.sync.dma_start(l0out[:, 2, bass.ts(i, 512)], ozt[:])

        # ---- scan helper ----
        # fwd and bwd run as two independent dependency chains (separate
        # psum/ACT/DVE/Pool instructions) so their per-step latencies
        # overlap. Gate order [i,f,o,g] with g-rows pre-scaled 2x on host:
        # one sigmoid covers all four slabs (tanh(g) = 2*sig(2g) - 1), and
        # tanh(c) = 2*sig(2c) - 1 via the activation input scale. Cell
        # update runs partly on the (otherwise idle) Pool engine.
        # h itself is OFF the critical cycle: the recurrent matmul takes
        # t2 = sig(o)*sig(2c) and sig(o) as moving operands against dual
        # stationaries [2*Whh^T | -Whh^T] (h = 2*t2 - sig(o)); the h tensor
        # for the next layer is materialized once per chunk in bulk.
        def scan(layer, mov, wst_d, kb_in, whhf_d, whhb_d, out_ap, kb_f, kb_b,
                 head=False):
            with ExitStack() as ctx:
                cpool = ctx.enter_context(tc.tile_pool(name=f"wh{layer}", bufs=1))
                whf = cpool.tile([128, 1024], BF16)
                whb = cpool.tile([128, 1024], BF16)
                idt = cpool.tile([128, 128], BF16)
                nc.sync.dma_start(whf[:], whhf_d[:])
                nc.sync.dma_start(whb[:], whhb_d[:])
                nc.sync.dma_start(idt[:], ident[:])
                wst = {}
                for d in "fb":
                    wst[d] = cpool.tile([128, kb_in, 512], BF16,
                                        name=f"wst{d}")
                    nc.sync.dma_start(wst[d][:], wst_d[d][:])
                if head:
                    lw_sb = cpool.tile([128, 2, 64], BF16)
                    lb_sb = cpool.tile([64, 1], F32)
                    w3_sb = cpool.tile([64, 1], BF16)
                    nc.sync.dma_start(lw_sb[:], lwk[:])
                    nc.sync.dma_start(lb_sb[:], lbv[:])
                    nc.sync.dma_start(w3_sb[:], w3s[:])

                ppool = ctx.enter_context(tc.tile_pool(name=f"pj{layer}", bufs=2))
                mvp = ctx.enter_context(tc.tile_pool(name=f"mv{layer}", bufs=2))
                prps = ctx.enter_context(
                    tc.tile_pool(name=f"prps{layer}", bufs=2, space="PSUM"))

                # JIT projection producer: computes the input projections for
                # fwd chunk ch / bwd chunk (n-1-ch) into SBUF while the scan
                # (latency-bound, PE ~idle) runs. Drains are emitted by the
                # caller spread across step slots to avoid blocking the
                # in-order DVE with big copies.
                def produce_loads(ch):
                    t0p, tb0p = ch * C, T - C - ch * C
                    mf = mvp.tile([128, kb_in, C * BL], BF16, tag="mf",
                                  name="mf")
                    mb = mvp.tile([128, kb_in, C * BL], BF16, tag="mb",
                                  name="mb")
                    nc.sync.dma_start(mf[:], mov[:, :, t0p * BL:(t0p + C) * BL])
                    nc.sync.dma_start(mb[:], mov[:, :, tb0p * BL:(tb0p + C) * BL])
                    pf = ppool.tile([128, 4, C * BL], BF16, tag="pf", name="pf")
                    pb = ppool.tile([128, 4, C * BL], BF16, tag="pb", name="pb")
                    return [pf, pb, mf, mb]

                def produce_group(pstate, k):
                    # one gate-block matmul group + its psum drain (~1.3us);
                    # emitted between step_pairs so it fills the cycle's idle
                    # PE/DVE windows instead of stalling the in-order queues
                    pf, pb, mf, mb = pstate
                    d, nt = ("f", k) if k < 4 else ("b", k - 4)
                    msb, dst = (mf, pf) if k < 4 else (mb, pb)
                    pp = prps.tile([128, C * BL], F32, tag="prp", name="prp")
                    for kb in range(kb_in):
                        nc.tensor.matmul(
                            pp[:], wst[d][:, kb, nt * 128:(nt + 1) * 128],
                            msb[:, kb, :], start=(kb == 0),
                            stop=(kb == kb_in - 1),
                            skip_group_check=True)
                    nc.vector.tensor_copy(dst[:, nt, :], pp[:])
                # head mode retains ALL chunk h-tiles so the output head can
                # consume time-chunk (ch, n-1-ch) pairs inside the loop
                hpool = ctx.enter_context(tc.tile_pool(
                    name=f"hc{layer}", bufs=(T // C) if head else 2))
                schp = {d: ctx.enter_context(
                    tc.tile_pool(name=f"sc{layer}{d}", bufs=2)) for d in "fb"}
                spool = {d: ctx.enter_context(
                    tc.tile_pool(name=f"s{layer}{d}", bufs=3)) for d in "fb"}
                cstp = {d: ctx.enter_context(
                    tc.tile_pool(name=f"cst{layer}{d}", bufs=2)) for d in "fb"}
                psum = {d: ctx.enter_context(
                    tc.tile_pool(name=f"ps{layer}{d}", bufs=3, space="PSUM"))
                    for d in "fb"}

                # per-dir state: (t2_prev_slice, o_prev_slice, cprev)
                st = {"f": (None, None, None), "b": (None, None, None)}
                wh = {"f": whf, "b": whb}

                # Stage-interleaved emission: each engine's instruction
                # stream alternates f/b so the in-order engines never queue
                # a dependent op of one chain ahead of the other chain's
                # ready op (which would serialize the chains).
                def step_pair(step, slots):
                    ps, S, t1, A, c1, cnew, s2c = ({} for _ in range(7))
                    for d, (proj_sl, Sch, t2ch, sl) in slots.items():
                        t2p, op_, _ = st[d]
                        ps[d] = psum[d].tile([128, 32], F32, name=f"ps{d}")
                        nc.tensor.matmul(ps[d][:], idt[:], proj_sl,
                                         start=True, stop=(step == 0),
                                         skip_group_check=True)
                        if step > 0:
                            for g in range(4):
                                nc.tensor.matmul(
                                    ps[d][:, g * 8:(g + 1) * 8],
                                    wh[d][:, g * 128:(g + 1) * 128], t2p,
                                    start=False, stop=False,
                                    skip_group_check=True)
                            for g in range(4):
                                nc.tensor.matmul(
                                    ps[d][:, g * 8:(g + 1) * 8],
                                    wh[d][:, 512 + g * 128:512 + (g + 1) * 128],
                                    op_, start=False, stop=(g == 3),
                                    skip_group_check=True)
                    for d, (proj_sl, Sch, t2ch, sl) in slots.items():
                        S[d] = Sch[:, sl, :]
                        nc.scalar.activation(S[d], ps[d][:], AF.Sigmoid)
                    # Cell-update trios run per-chain back-to-back on DVE
                    # (f's three ops, then b's): an on-cycle Pool op costs
                    # ~260ns (dispatch + Q7 launch) vs ~155ns on DVE, and
                    # per-chain grouping keeps the in-order DVE from gating
                    # one chain on the other's sigmoid.
                    for d in slots:
                        _, _, cprev = st[d]
                        # A/2 = (sig(2g) - 0.5) * sig(i)  [= sig(i)*tanh(g)/2]
                        t1[d] = spool[d].tile([128, 8], F32, tag="t1", name=f"t1{d}")
                        nc.vector.scalar_tensor_tensor(
                            t1[d][:], S[d][:, 24:32], -0.5, S[d][:, 0:8],
                            OP.add, OP.mult)
                        cnew[d] = cstp[d].tile([128, 8], F32, tag="c", name=f"c{d}")
                        if step > 0:
                            c1[d] = cstp[d].tile([128, 8], F32, tag="c1", name=f"c1{d}")
                            nc.vector.tensor_tensor(c1[d][:], S[d][:, 8:16],
                                                    cprev[:], OP.mult)
                            nc.vector.scalar_tensor_tensor(
                                cnew[d][:], t1[d][:], 2.0, c1[d][:],
                                OP.mult, OP.add)
                        else:
                            nc.vector.tensor_scalar(
                                cnew[d][:], t1[d][:], 2.0, None, OP.mult)
                    for d in slots:
                        s2c[d] = spool[d].tile([128, 8], F32, tag="s2c", name=f"s2c{d}")
                        nc.scalar.activation(s2c[d][:], cnew[d][:],
                                             AF.Sigmoid, scale=2.0)
                    for d, (proj_sl, Sch, t2ch, sl) in slots.items():
                        t2sl = t2ch[:, sl, :]
                        nc.vector.tensor_tensor(t2sl, S[d][:, 16:24],
                                                s2c[d][:], OP.mult)
                        st[d] = (t2sl, S[d][:, 16:24], cnew[d])

                # head: u = tanh(LW @ [h_f; h_b] + LB), du = w3s . u,
                # scattered to dud[tc]; consumes retained SBUF h-tiles.
                ht_f, ht_b = {}, {}

                def head_consume(tc_):
                    pu = prps.tile([128, C * BL], F32, tag="prp", name="hpu")
                    nc.tensor.matmul(pu[0:64, :], lw_sb[:, 0, :],
                                     ht_f[tc_][:], start=True, stop=False,
                                     skip_group_check=True)
                    nc.tensor.matmul(pu[0:64, :], lw_sb[:, 1, :],
                                     ht_b[tc_][:], start=False, stop=True,
                                     skip_group_check=True)
                    ut = spool["f"].tile([64, C * BL], BF16, tag="hut",
                                         name="hut")
                    nc.scalar.activation(ut[:], pu[0:64, :], AF.Tanh,
                                         bias=lb_sb[:])
                    pd = prps.tile([128, C * BL], F32, tag="prp", name="hpd")
                    nc.tensor.matmul(pd[0:1, :], w3_sb[:], ut[:],
                                     start=True, stop=True,
                                     skip_group_check=True)
                    dt_ = spool["f"].tile([1, C * BL], F32, tag="hdt",
                                          name="hdt")
                    nc.vector.tensor_copy(dt_[:], pd[0:1, :])
                    src = dt_[:].rearrange("o (t b) -> o t b", b=BL)
                    dst = dud[tc_:tc_ + 1].rearrange("o b t -> o t b")
                    nc.sync.dma_start(dst, src)

                n_chunks = T // C
                cur = produce_loads(0)
                for k in range(8):
                    produce_group(cur, k)
                pf, pb = cur[0], cur[1]
                for ch in range(n_chunks):
                    t0 = ch * C
                    tb0 = T - C - t0  # bwd chunk start (ascending t)
                    nxt = produce_loads(ch + 1) if ch + 1 < n_chunks else None
                    Sf_ch = schp["f"].tile([128, C, 32], BF16, tag="Sch")
                    Sb_ch = schp["b"].tile([128, C, 32], BF16, tag="Sch")
                    t2f_ch = schp["f"].tile([128, C, 8], BF16, tag="t2ch")
                    t2b_ch = schp["b"].tile([128, C, 8], BF16, tag="t2ch")
                    hf_ch = hpool.tile([128, C * BL], BF16, tag="hf")
                    hb_ch = hpool.tile([128, C * BL], BF16, tag="hb")

                    for c in range(C):
                        step = t0 + c
                        cb = C - 1 - c  # bwd slot (reversed within chunk)
                        step_pair(step, {
                            "f": (pf[:, :, c * BL:(c + 1) * BL], Sf_ch, t2f_ch, c),
                            "b": (pb[:, :, cb * BL:(cb + 1) * BL], Sb_ch, t2b_ch, cb),
                        })
                        if nxt is not None and c % 8 == 4:
                            produce_group(nxt, c // 8)

                    # bulk h = 2*t2 - sig(o) for the whole chunk (off-cycle)
                    hf_v = hf_ch[:].rearrange("p (c x) -> p c x", x=BL)
                    hb_v = hb_ch[:].rearrange("p (c x) -> p c x", x=BL)
                    nc.vector.scalar_tensor_tensor(
                        hf_v, t2f_ch[:], 2.0, Sf_ch[:, :, 16:24],
                        OP.mult, OP.subtract)
                    nc.vector.scalar_tensor_tensor(
                        hb_v, t2b_ch[:], 2.0, Sb_ch[:, :, 16:24],
                        OP.mult, OP.subtract)
                    if out_ap is not None:
                        nc.sync.dma_start(
                            out_ap[:, kb_f, t0 * BL:(t0 + C) * BL], hf_ch[:])
                        nc.sync.dma_start(
                            out_ap[:, kb_b, tb0 * BL:(tb0 + C) * BL], hb_ch[:])
                    if head:
                        ht_f[ch] = hf_ch
                        ht_b[n_chunks - 1 - ch] = hb_ch
                        if ch >= n_chunks // 2:
                            # time-chunks ch and n-1-ch just became complete
                            head_consume(ch)
                            head_consume(n_chunks - 1 - ch)
                    if nxt is not None:
                        pf, pb = nxt[0], nxt[1]

        # ---- S0 (with JIT layer-0 input projections) ----
        scan(0, xk, w0, 4, whh0["f"], whh0["b"], l0out, 0, 1)

        # ---- S1 (with JIT layer-1 input projections) ----
        scan(1, l0out, w1, 3, whh1["f"], whh1["b"], None, 0, 1,
             head=True)

        # ---- P4: context solve (jacobi) + output ----
        # Packed [b*16 + blk, w] across all 128 partitions, with a HALO-col
        # redundant left margin per block: influence travels 2 cols/iter, so
        # HALO=20 > 2*N_JACOBI keeps block seams out of the final region.
        # Seq-start rows (p % 16 == 0) mask their halo (and its softplus) to
        # zero, reproducing the d=0, sp=0 start condition.
        HALO, EXT = 20, 20 + W4
        with ExitStack() as ctx:
            cpool = ctx.enter_context(tc.tile_pool(name="ctxc", bufs=1))
            cf = cpool.tile([128, 8], F32)
            nc.sync.dma_start(cf[:], coef[:])
            mskt = cpool.tile([128, HALO], F32)
            nc.sync.dma_start(mskt[:], cmsk[:])
            d0x = cpool.tile([128, EXT], F32)
            nc.vector.memset(d0x[:, 0:HALO], 0.0)
            for b in range(BL):
                # partition b*16+k holds block k of sequence b
                nc.sync.dma_start(d0x[b * 16:(b + 1) * 16, HALO:EXT],
                                  dud[:, b, :])
                # halo: last HALO cols of block k-1 (rows b*16+1..b*16+15;
                # seq-start rows p=b*16 keep the memset zeros)
                nc.sync.dma_start(d0x[b * 16 + 1:(b + 1) * 16, 0:HALO],
                                  dud[0:15, b, W4 - HALO:W4])
            # += K0 everywhere, then re-zero seq-start halos (K0 was added)
            nc.vector.tensor_scalar(d0x[:], d0x[:], cf[:, 4:5], None, OP.add)
            nc.vector.tensor_tensor(d0x[:, 0:HALO], d0x[:, 0:HALO], mskt[:],
                                    OP.mult)
            jp = ctx.enter_context(tc.tile_pool(name="jac", bufs=2))
            sp_p = ctx.enter_context(tc.tile_pool(name="jsp", bufs=2))
            d_cur = d0x
            g_, dl_, a_, b_ = (cf[:, 0:1], cf[:, 1:2], cf[:, 2:3], cf[:, 3:4])

            def stt(out, in0, scal, in1):
                nc.vector.scalar_tensor_tensor(out, in0, scal, in1,
                                               OP.mult, OP.add)

            def softplus(out_ap, in_ap):
                # Softplus has no ACT table on this build: ln(1 + exp(x)),
                # with the +1 folded into Ln's constant bias (no DVE hop).
                # d stays small (|d| < ~3) so no overflow concerns.
                nc.scalar.activation(out_ap, in_ap, AF.Exp)
                nc.scalar.activation(out_ap, out_ap, AF.Ln, bias=1.0)

            for it in range(N_JACOBI):
                sp = sp_p.tile([128, EXT], F32, tag="sp")
                softplus(sp[:], d_cur[:])
                nc.vector.tensor_tensor(sp[:, 0:HALO], sp[:, 0:HALO],
                                        mskt[:], OP.mult)
                acc = jp.tile([128, EXT], F32, tag="acc")
                nc.vector.tensor_copy(acc[:, 0:2], d0x[:, 0:2])
                stt(acc[:, 1:EXT], d_cur[:, 0:EXT - 1], g_, d0x[:, 1:EXT])
                stt(acc[:, 1:EXT], sp[:, 0:EXT - 1], dl_, acc[:, 1:EXT])
                stt(acc[:, 2:EXT], d_cur[:, 0:EXT - 2], a_, acc[:, 2:EXT])
                stt(acc[:, 2:EXT], sp[:, 0:EXT - 2], b_, acc[:, 2:EXT])
                d_cur = acc

            spf = sp_p.tile([128, EXT], F32, tag="sp")
            softplus(spf[:], d_cur[:])
            lo = cpool.tile([128, W4, 2], F32)
            nc.vector.tensor_scalar(lo[:, :, 0], spf[:, HALO:EXT], -1.0,
                                    None, OP.mult)
            nc.vector.tensor_tensor(lo[:, :, 1], d_cur[:, HALO:EXT],
                                    spf[:, HALO:EXT], OP.subtract)
            out_view = outv.rearrange("(b k w) x -> (b k) w x", b=BL, k=16)
            nc.sync.dma_start(out_view, lo[:])

    nc.compile()
    _BUILD_CACHE[T] = nc
    return nc


# ---------------------------------------------------------------------------
# host-side prep + execution
# ---------------------------------------------------------------------------
def _prep_shared(inputs):
    sh = {}
    for l, (din_, kpad, wkey) in enumerate(((D_IN, 512, "w0"),
                                            (256, 384, "w1"))):
        for d, suf in (("f", ""), ("b", "r")):
            wih = _perm_gates(inputs[f"w_ih_l{l}{suf}"])       # [512, din]
            whh = _perm_gates(inputs[f"w_hh_l{l}{suf}"])       # [512, 128]
            bias = _perm_gates(
                np.asarray(inputs[f"b_ih_l{l}{suf}"])
                + np.asarray(inputs[f"b_hh_l{l}{suf}"]))       # [512]
            aug = np.zeros((kpad, 512), np.float32)
            aug[:din_] = np.asarray(wih, np.float32).T
            aug[din_] = bias
            sh[f"{wkey}{d}"] = _bf16(
                aug.reshape(kpad // 128, 128, 512).transpose(1, 0, 2))
            wT = np.asarray(whh, np.float32).T          # [128, 512]
            sh[f"whh{l}{d}"] = _bf16(
                np.concatenate([2.0 * wT, -wT], axis=1))  # h = 2*t2 - sig(o)

    g1, b1 = np.asarray(inputs["bn1_g"]), np.asarray(inputs["bn1_b"])
    m1, v1 = np.asarray(inputs["bn1_m"]), np.asarray(inputs["bn1_v"])
    s1 = g1 / np.sqrt(v1 + EPS)
    t1 = b1 - m1 * s1
    lin_w = np.asarray(inputs["lin_w"])
    LW = lin_w * s1[None, :]
    LB = np.asarray(inputs["lin_b"]) + lin_w @ t1
    g2, b2 = np.asarray(inputs["bn2_g"]), np.asarray(inputs["bn2_b"])
    m2, v2 = np.asarray(inputs["bn2_m"]), np.asarray(inputs["bn2_v"])
    s2 = g2 / np.sqrt(v2 + EPS)
    t2 = b2 - m2 * s2
    out_w, out_b = np.asarray(inputs["out_w"]), np.asarray(inputs["out_b"])
    W1, W2, W3 = out_w[:, 0:2], out_w[:, 2:4], out_w[:, 4:68]
    w3d = W3[1] - W3[0]
    K0 = (out_b[1] - out_b[0]) + t2 @ w3d
    w1d, w2d = W1[1] - W1[0], W2[1] - W2[0]
    alpha, beta = w1d[1], w1d[0] + w1d[1]
    gamma, delta = w2d[1], w2d[0] + w2d[1]

    sh["lwk"] = _bf16(LW.T.reshape(2, 128, 64).transpose(1, 0, 2))
    sh["lbv"] = np.asarray(LB, np.float32).reshape(64, 1)
    sh["w3s"] = _bf16((w3d * s2).reshape(64, 1))
    coefs = np.zeros((128, 8), np.float32)
    coefs[:, 0] = gamma
    coefs[:, 1] = -delta
    coefs[:, 2] = alpha
    coefs[:, 3] = -beta
    coefs[:, 4] = K0
    sh["coef"] = coefs
    cmsk = np.ones((128, 20), np.float32)
    cmsk[0::16, :] = 0.0          # sequence-start rows: zero halo
    sh["cmsk"] = cmsk
    sh["ident"] = _bf16(np.eye(128, dtype=np.float32))
    return sh


def _prep_core(x_core):
    # x_core: [BL, T, 400] -> aug kxn [128, 4, T*BL] bf16
    xt = np.zeros((512, T * BL), np.float32)
    xt[:D_IN] = np.asarray(x_core, np.float32).transpose(2, 1, 0).reshape(
        D_IN, T * BL)
    xt[D_IN] = 1.0
    return _bf16(xt.reshape(4, 128, T * BL).transpose(1, 0, 2))


def kernel(**inputs):
    nc = _build()
    sh = _prep_shared(inputs)
    x = np.asarray(inputs["x"], np.float32)
    in_maps = []
    for cidx in range(N_CORES):
        m = dict(sh)
        m["xk"] = _prep_core(x[cidx * BL:(cidx + 1) * BL])
        in_maps.append(m)
    res = run_bass_kernel_spmd(nc, in_maps, list(range(N_CORES)))
    outs = [np.asarray(res.results[i]["outv"], np.float32)
            for i in range(N_CORES)]
    return np.concatenate(outs, axis=0)


if __name__ == "__main__":
    import time
    t0 = time.time()
    print(f"building T={T}...")
    _build()
    print(f"built in {time.time() - t0:.1f}s")



# revision 47
# speedup vs baseline: 1.1759x; 1.0604x over previous
"""Trainium2 Bass kernel for nn_BiLSTMWithLM (B=64, T=1024, D_IN=400).

Data-parallel over batch: 8 cores x 8 sequences each.
  S0/S1: bidirectional LSTM scans (layer 0 then layer 1). fwd and bwd run
      as two independent dependency chains whose per-step latencies overlap.
      Input projections are produced just-in-time inside the scan (JIT
      producer matmul groups + psum drains spread across step slots) —
      there is no separate projection phase and no proj DRAM round trip.
      Per step: identity-matmul preloads PSUM with the input projection,
      recurrent matmuls accumulate against dual stationaries
      [2*Whh^T | -Whh^T] taking t2 = sig(o)*sig(2c) and sig(o) as moving
      operands (h = 2*t2 - sig(o) stays OFF the critical cycle; h for the
      next layer is materialized once per chunk in bulk). Gate order
      [i,f,o,g] with g-rows pre-scaled 2x on host: ONE sigmoid covers all
      four gate slabs (tanh(g) = 2*sig(2g) - 1), and tanh(c) = 2*sig(2c)-1
      via the activation input scale. Cell update: A/2 = (sig(2g)-0.5)*
      sig(i) in one DVE op, then c = 2*(A/2) + f*c_prev; the three DVE ops
      run per-chain back-to-back (an on-cycle Pool op costs ~260ns vs
      ~155ns on DVE due to dispatch + Q7 launch).
  head (inside S1): BN1/linear/BN2 folded on host into LW/LB; u =
      tanh(LW @ [h_f; h_b] + LB) and the logit-difference drive
      du = w3s . u are computed per time-chunk from SBUF-retained h tiles
      during S1 iterations ch >= 8 (when both directions of time-chunks
      ch and 15-ch are complete), scattered to dud[blk, b, w].
  P4: context scan reformulated as a scalar recurrence on the logit diff
      d_t = du_t + g*d_{t-1} - dl*sp(d_{t-1}) + a*d_{t-2} - b*sp(d_{t-2}),
      solved by Jacobi fixed-point iteration (contraction ~0.085/iter)
      vectorized over [b*16+blk, w] partitions with a 20-col redundant
      halo (influence travels 2 cols/iter so block seams never reach the
      final region); lo0 = -softplus(d), lo1 = d - softplus(d).
"""
import os
import sys

sys.path.insert(0, "/opt/trn_rl_repo")

import numpy as np
import ml_dtypes

import concourse.bass as bass
import concourse.bacc as bacc
import concourse.mybir as mybir
from concourse import tile
from concourse.bass_utils import run_bass_kernel_spmd
from concourse.kernels.tile_matmul import matmul_tile_kernel
from contextlib import ExitStack

BF16 = mybir.dt.bfloat16
F32 = mybir.dt.float32
AF = mybir.ActivationFunctionType
OP = mybir.AluOpType

B, D_IN, H = 64, 400, 128
T = int(os.environ.get("KERNEL_T", "1024"))
N_CORES = 8
BL = B // N_CORES          # 8 local sequences
N = T * BL                 # columns, n = t*8 + b
C = 64                     # scan chunk (steps per DMA chunk)
N_JACOBI = 5
EPS = 1e-5


def _bf16(x):
    return np.asarray(x, dtype=ml_dtypes.bfloat16)


def _perm_gates(w):
    # torch order [i,f,g,o] -> [i,f,o,g], with g rows scaled 2x so the
    # kernel can use tanh(g) = 2*sigmoid(2g) - 1 with a single sigmoid.
    i, f, g, o = np.split(np.asarray(w), 4, axis=0)
    return np.concatenate([i, f, o, 2.0 * g], axis=0)


_BUILD_CACHE = {}


def _build():
    if T in _BUILD_CACHE:
        return _BUILD_CACHE[T]

    nc = bacc.Bacc("TRN2", target_bir_lowering=False, debug=False,
                   num_devices=N_CORES)

    def din(name, shape, dtype):
        return nc.dram_tensor(name, shape, dtype, kind="ExternalInput").ap()

    def dscratch(name, shape, dtype):
        return nc.dram_tensor(name, shape, dtype).ap()

    # inputs
    xk = din("xk", [128, 4, N], BF16)               # aug x, kxn for P1
    w0 = {d: din(f"w0{d}", [128, 4, 512], BF16) for d in "fb"}
    w1 = {d: din(f"w1{d}", [128, 3, 512], BF16) for d in "fb"}
    whh0 = {d: din(f"whh0{d}", [128, 1024], BF16) for d in "fb"}
    whh1 = {d: din(f"whh1{d}", [128, 1024], BF16) for d in "fb"}
    lwk = din("lwk", [128, 2, 64], BF16)            # LW.T tiled
    lbv = din("lbv", [64, 1], F32)                  # LB bias
    w3s = din("w3s", [64, 1], BF16)                 # head diff vector
    coef = din("coef", [128, 8], F32)               # [g, -dl, a, -b, K0]
    cmsk = din("cmsk", [128, 20], F32)              # 0 at seq-start rows
    ident = din("ident", [128, 128], BF16)
    outv = nc.dram_tensor("outv", [N, 2], F32, kind="ExternalOutput").ap()

    # scratch
    l0out = dscratch("l0out", [128, 3, N], BF16)
    # du as [blk, b, w]: time-block-major so the P3 scatter is a plain
    # first-axis index (clean DMA region for dependency tracking)
    W4 = T // 16
    dud = dscratch("dud", [16, BL, W4], F32)

    with tile.TileContext(nc) as tc:
        # ---- init: l0out kb=2 block (ones row at p=0, zeros elsewhere) ----
        with ExitStack() as ctx:
            pool = ctx.enter_context(tc.tile_pool(name="initp", bufs=1))
            ozt = pool.tile([128, 512], BF16)
            nc.vector.memset(ozt[:], 0.0)
            nc.vector.memset(ozt[0:1, :], 1.0)
            for i in range(N // 512):
                nc.sync.dma_start(l0out[:, 2, bass.ts(i, 512)], ozt[:])

        # ---- scan helper ----
        # fwd and bwd run as two independent dependency chains (separate
        # psum/ACT/DVE/Pool instructions) so their per-step latencies
        # overlap. Gate order [i,f,o,g] with g-rows pre-scaled 2x on host:
        # one sigmoid covers all four slabs (tanh(g) = 2*sig(2g) - 1), and
        # tanh(c) = 2*sig(2c) - 1 via the activation input scale. Cell
        # update runs partly on the (otherwise idle) Pool engine.
        # h itself is OFF the critical cycle: the recurrent matmul takes
        # t2 = sig(o)*sig(2c) and sig(o) as moving operands against dual
        # stationaries [2*Whh^T | -Whh^T] (h = 2*t2 - sig(o)); the h tensor
        # for the next layer is materialized once per chunk in bulk.
        def scan(layer, mov, wst_d, kb_in, whhf_d, whhb_d, out_ap, kb_f, kb_b,
                 head=False):
            with ExitStack() as ctx:
                cpool = ctx.enter_context(tc.tile_pool(name=f"wh{layer}", bufs=1))
                whf = cpool.tile([128, 1024], BF16)
                whb = cpool.tile([128, 1024], BF16)
                idt = cpool.tile([128, 128], BF16)
                nc.sync.dma_start(whf[:], whhf_d[:])
                nc.sync.dma_start(whb[:], whhb_d[:])
                nc.sync.dma_start(idt[:], ident[:])
                wst = {}
                for d in "fb":
                    wst[d] = cpool.tile([128, kb_in, 512], BF16,
                                        name=f"wst{d}")
                    nc.sync.dma_start(wst[d][:], wst_d[d][:])
                if head:
                    lw_sb = cpool.tile([128, 2, 64], BF16)
                    lb_sb = cpool.tile([64, 1], F32)
                    w3_sb = cpool.tile([64, 1], BF16)
                    nc.sync.dma_start(lw_sb[:], lwk[:])
                    nc.sync.dma_start(lb_sb[:], lbv[:])
                    nc.sync.dma_start(w3_sb[:], w3s[:])

                ppool = ctx.enter_context(tc.tile_pool(name=f"pj{layer}", bufs=2))
                mvp = ctx.enter_context(tc.tile_pool(name=f"mv{layer}", bufs=2))
                prps = ctx.enter_context(
                    tc.tile_pool(name=f"prps{layer}", bufs=2, space="PSUM"))

                # JIT projection producer: computes the input projections for
                # fwd chunk ch / bwd chunk (n-1-ch) into SBUF while the scan
                # (latency-bound, PE ~idle) runs. Drains are emitted by the
                # caller spread across step slots to avoid blocking the
                # in-order DVE with big copies.
                def produce_loads(ch):
                    t0p, tb0p = ch * C, T - C - ch * C
                    mf = mvp.tile([128, kb_in, C * BL], BF16, tag="mf",
                                  name="mf")
                    mb = mvp.tile([128, kb_in, C * BL], BF16, tag="mb",
                                  name="mb")
                    nc.sync.dma_start(mf[:], mov[:, :, t0p * BL:(t0p + C) * BL])
                    nc.sync.dma_start(mb[:], mov[:, :, tb0p * BL:(tb0p + C) * BL])
                    pf = ppool.tile([128, 4, C * BL], BF16, tag="pf", name="pf")
                    pb = ppool.tile([128, 4, C * BL], BF16, tag="pb", name="pb")
                    return [pf, pb, mf, mb]

                def produce_group(pstate, k):
                    # one gate-block matmul group + its psum drain (~1.3us);
                    # emitted between step_pairs so it fills the cycle's idle
                    # PE/DVE windows instead of stalling the in-order queues
                    pf, pb, mf, mb = pstate
                    d, nt = ("f", k) if k < 4 else ("b", k - 4)
                    msb, dst = (mf, pf) if k < 4 else (mb, pb)
                    pp = prps.tile([128, C * BL], F32, tag="prp", name="prp")
                    for kb in range(kb_in):
                        nc.tensor.matmul(
                            pp[:], wst[d][:, kb, nt * 128:(nt + 1) * 128],
                            msb[:, kb, :], start=(kb == 0),
                            stop=(kb == kb_in - 1),
                            skip_group_check=True)
                    nc.vector.tensor_copy(dst[:, nt, :], pp[:])
                # head mode retains ALL chunk h-tiles so the output head can
                # consume time-chunk (ch, n-1-ch) pairs inside the loop
                hpool = ctx.enter_context(tc.tile_pool(
                    name=f"hc{layer}", bufs=(T // C) if head else 2))
                schp = {d: ctx.enter_context(
                    tc.tile_pool(name=f"sc{layer}{d}", bufs=2)) for d in "fb"}
                spool = {d: ctx.enter_context(
                    tc.tile_pool(name=f"s{layer}{d}", bufs=3)) for d in "fb"}
                cstp = {d: ctx.enter_context(
                    tc.tile_pool(name=f"cst{layer}{d}", bufs=2)) for d in "fb"}
                psum = {d: ctx.enter_context(
                    tc.tile_pool(name=f"ps{layer}{d}", bufs=3, space="PSUM"))
                    for d in "fb"}

                # per-dir state: (t2_prev_slice, o_prev_slice, cprev)
                st = {"f": (None, None, None), "b": (None, None, None)}
                wh = {"f": whf, "b": whb}

                # Stage-interleaved emission: each engine's instruction
                # stream alternates f/b so the in-order engines never queue
                # a dependent op of one chain ahead of the other chain's
                # ready op (which would serialize the chains).
                def step_pair(step, slots):
                    ps, S, t1, A, c1, cnew, s2c = ({} for _ in range(7))
                    for d, (proj_sl, Sch, t2ch, sl) in slots.items():
                        t2p, op_, _ = st[d]
                        ps[d] = psum[d].tile([128, 32], F32, name=f"ps{d}")
                        nc.tensor.matmul(ps[d][:], idt[:], proj_sl,
                                         start=True, stop=(step == 0),
                                         skip_group_check=True)
                        if step > 0:
                            # sig(o)-mms first: their moving operand is ready
                            # ~1us before t2, so the in-order PE retires them
                            # (and their stationary loads) off the wait path,
                            # leaving only the 4 t2-mms after t2 arrives.
                            for g in range(4):
                                nc.tensor.matmul(
                                    ps[d][:, g * 8:(g + 1) * 8],
                                    wh[d][:, 512 + g * 128:512 + (g + 1) * 128],
                                    op_, start=False, stop=False,
                                    skip_group_check=True)
                            for g in range(4):
                                nc.tensor.matmul(
                                    ps[d][:, g * 8:(g + 1) * 8],
                                    wh[d][:, g * 128:(g + 1) * 128], t2p,
                                    start=False, stop=(g == 3),
                                    skip_group_check=True)
                    for d, (proj_sl, Sch, t2ch, sl) in slots.items():
                        S[d] = Sch[:, sl, :]
                        nc.scalar.activation(S[d], ps[d][:], AF.Sigmoid)
                    # Cell-update trios run per-chain back-to-back on DVE
                    # (f's three ops, then b's): an on-cycle Pool op costs
                    # ~260ns (dispatch + Q7 launch) vs ~155ns on DVE, and
                    # per-chain grouping keeps the in-order DVE from gating
                    # one chain on the other's sigmoid.
                    for d in slots:
                        _, _, cprev = st[d]
                        # A/2 = (sig(2g) - 0.5) * sig(i)  [= sig(i)*tanh(g)/2]
                        t1[d] = spool[d].tile([128, 8], F32, tag="t1", name=f"t1{d}")
                        nc.vector.scalar_tensor_tensor(
                            t1[d][:], S[d][:, 24:32], -0.5, S[d][:, 0:8],
                            OP.add, OP.mult)
                        cnew[d] = cstp[d].tile([128, 8], F32, tag="c", name=f"c{d}")
                        if step > 0:
                            c1[d] = cstp[d].tile([128, 8], F32, tag="c1", name=f"c1{d}")
                            nc.vector.tensor_tensor(c1[d][:], S[d][:, 8:16],
                                                    cprev[:], OP.mult)
                            nc.vector.scalar_tensor_tensor(
                                cnew[d][:], t1[d][:], 2.0, c1[d][:],
                                OP.mult, OP.add)
                        else:
                            nc.vector.tensor_scalar(
                                cnew[d][:], t1[d][:], 2.0, None, OP.mult)
                    for d in slots:
                        s2c[d] = spool[d].tile([128, 8], F32, tag="s2c", name=f"s2c{d}")
                        nc.scalar.activation(s2c[d][:], cnew[d][:],
                                             AF.Sigmoid, scale=2.0)
                    for d, (proj_sl, Sch, t2ch, sl) in slots.items():
                        t2sl = t2ch[:, sl, :]
                        nc.vector.tensor_tensor(t2sl, S[d][:, 16:24],
                                                s2c[d][:], OP.mult)
                        st[d] = (t2sl, S[d][:, 16:24], cnew[d])

                # head: u = tanh(LW @ [h_f; h_b] + LB), du = w3s . u,
                # scattered to dud[tc]; consumes retained SBUF h-tiles.
                ht_f, ht_b = {}, {}

                def head_consume(tc_):
                    pu = prps.tile([128, C * BL], F32, tag="prp", name="hpu")
                    nc.tensor.matmul(pu[0:64, :], lw_sb[:, 0, :],
                                     ht_f[tc_][:], start=True, stop=False,
                                     skip_group_check=True)
                    nc.tensor.matmul(pu[0:64, :], lw_sb[:, 1, :],
                                     ht_b[tc_][:], start=False, stop=True,
                                     skip_group_check=True)
                    ut = spool["f"].tile([64, C * BL], BF16, tag="hut",
                                         name="hut")
                    nc.scalar.activation(ut[:], pu[0:64, :], AF.Tanh,
                                         bias=lb_sb[:])
                    pd = prps.tile([128, C * BL], F32, tag="prp", name="hpd")
                    nc.tensor.matmul(pd[0:1, :], w3_sb[:], ut[:],
                                     start=True, stop=True,
                                     skip_group_check=True)
                    dt_ = spool["f"].tile([1, C * BL], F32, tag="hdt",
                                          name="hdt")
                    nc.vector.tensor_copy(dt_[:], pd[0:1, :])
                    src = dt_[:].rearrange("o (t b) -> o t b", b=BL)
                    dst = dud[tc_:tc_ + 1].rearrange("o b t -> o t b")
                    nc.sync.dma_start(dst, src)

                n_chunks = T // C
                cur = produce_loads(0)
                for k in range(8):
                    produce_group(cur, k)
                pf, pb = cur[0], cur[1]
                for ch in range(n_chunks):
                    t0 = ch * C
                    tb0 = T - C - t0  # bwd chunk start (ascending t)
                    nxt = produce_loads(ch + 1) if ch + 1 < n_chunks else None
                    Sf_ch = schp["f"].tile([128, C, 32], BF16, tag="Sch")
                    Sb_ch = schp["b"].tile([128, C, 32], BF16, tag="Sch")
                    t2f_ch = schp["f"].tile([128, C, 8], BF16, tag="t2ch")
                    t2b_ch = schp["b"].tile([128, C, 8], BF16, tag="t2ch")
                    hf_ch = hpool.tile([128, C * BL], BF16, tag="hf")
                    hb_ch = hpool.tile([128, C * BL], BF16, tag="hb")

                    for c in range(C):
                        step = t0 + c
                        cb = C - 1 - c  # bwd slot (reversed within chunk)
                        step_pair(step, {
                            "f": (pf[:, :, c * BL:(c + 1) * BL], Sf_ch, t2f_ch, c),
                            "b": (pb[:, :, cb * BL:(cb + 1) * BL], Sb_ch, t2b_ch, cb),
                        })
                        if nxt is not None and c % 8 == 4:
                            produce_group(nxt, c // 8)

                    # bulk h = 2*t2 - sig(o) for the whole chunk (off-cycle)
                    hf_v = hf_ch[:].rearrange("p (c x) -> p c x", x=BL)
                    hb_v = hb_ch[:].rearrange("p (c x) -> p c x", x=BL)
                    nc.vector.scalar_tensor_tensor(
                        hf_v, t2f_ch[:], 2.0, Sf_ch[:, :, 16:24],
                        OP.mult, OP.subtract)
                    nc.vector.scalar_tensor_tensor(
                        hb_v, t2b_ch[:], 2.0, Sb_ch[:, :, 16:24],
                        OP.mult, OP.subtract)
                    if out_ap is not None:
                        nc.sync.dma_start(
                            out_ap[:, kb_f, t0 * BL:(t0 + C) * BL], hf_ch[:])
                        nc.sync.dma_start(
                            out_ap[:, kb_b, tb0 * BL:(tb0 + C) * BL], hb_ch[:])
                    if head:
                        ht_f[ch] = hf_ch
                        ht_b[n_chunks - 1 - ch] = hb_ch
                        if ch >= n_chunks // 2:
                            # time-chunks ch and n-1-ch just became complete
                            head_consume(ch)
                            head_consume(n_chunks - 1 - ch)
                    if nxt is not None:
                        pf, pb = nxt[0], nxt[1]

        # ---- S0 (with JIT layer-0 input projections) ----
        scan(0, xk, w0, 4, whh0["f"], whh0["b"], l0out, 0, 1)

        # ---- S1 (with JIT layer-1 input projections) ----
        scan(1, l0out, w1, 3, whh1["f"], whh1["b"], None, 0, 1,
             head=True)

        # ---- P4: context solve (jacobi) + output ----
        # Packed [b*16 + blk, w] across all 128 partitions, with a HALO-col
        # redundant left margin per block: influence travels 2 cols/iter, so
        # HALO=20 > 2*N_JACOBI keeps block seams out of the final region.
        # Seq-start rows (p % 16 == 0) mask their halo (and its softplus) to
        # zero, reproducing the d=0, sp=0 start condition.
        HALO, EXT = 20, 20 + W4
        with ExitStack() as ctx:
            cpool = ctx.enter_context(tc.tile_pool(name="ctxc", bufs=1))
            cf = cpool.tile([128, 8], F32)
            nc.sync.dma_start(cf[:], coef[:])
            mskt = cpool.tile([128, HALO], F32)
            nc.sync.dma_start(mskt[:], cmsk[:])
            d0x = cpool.tile([128, EXT], F32)
            nc.vector.memset(d0x[:, 0:HALO], 0.0)
            for b in range(BL):
                # partition b*16+k holds block k of sequence b
                nc.sync.dma_start(d0x[b * 16:(b + 1) * 16, HALO:EXT],
                                  dud[:, b, :])
                # halo: last HALO cols of block k-1 (rows b*16+1..b*16+15;
                # seq-start rows p=b*16 keep the memset zeros)
                nc.sync.dma_start(d0x[b * 16 + 1:(b + 1) * 16, 0:HALO],
                                  dud[0:15, b, W4 - HALO:W4])
            # += K0 everywhere, then re-zero seq-start halos (K0 was added)
            nc.vector.tensor_scalar(d0x[:], d0x[:], cf[:, 4:5], None, OP.add)
            nc.vector.tensor_tensor(d0x[:, 0:HALO], d0x[:, 0:HALO], mskt[:],
                                    OP.mult)
            jp = ctx.enter_context(tc.tile_pool(name="jac", bufs=2))
            sp_p = ctx.enter_context(tc.tile_pool(name="jsp", bufs=2))
            d_cur = d0x
            g_, dl_, a_, b_ = (cf[:, 0:1], cf[:, 1:2], cf[:, 2:3], cf[:, 3:4])

            def stt(out, in0, scal, in1):
                nc.vector.scalar_tensor_tensor(out, in0, scal, in1,
                                               OP.mult, OP.add)

            def softplus(out_ap, in_ap):
                # Softplus has no ACT table on this build: ln(1 + exp(x)),
                # with the +1 folded into Ln's constant bias (no DVE hop).
                # d stays small (|d| < ~3) so no overflow concerns.
                nc.scalar.activation(out_ap, in_ap, AF.Exp)
                nc.scalar.activation(out_ap, out_ap, AF.Ln, bias=1.0)

            for it in range(N_JACOBI):
                sp = sp_p.tile([128, EXT], F32, tag="sp")
                softplus(sp[:], d_cur[:])
                nc.vector.tensor_tensor(sp[:, 0:HALO], sp[:, 0:HALO],
                                        mskt[:], OP.mult)
                acc = jp.tile([128, EXT], F32, tag="acc")
                nc.vector.tensor_copy(acc[:, 0:2], d0x[:, 0:2])
                stt(acc[:, 1:EXT], d_cur[:, 0:EXT - 1], g_, d0x[:, 1:EXT])
                stt(acc[:, 1:EXT], sp[:, 0:EXT - 1], dl_, acc[:, 1:EXT])
                stt(acc[:, 2:EXT], d_cur[:, 0:EXT - 2], a_, acc[:, 2:EXT])
                stt(acc[:, 2:EXT], sp[:, 0:EXT - 2], b_, acc[:, 2:EXT])
                d_cur = acc

            spf = sp_p.tile([128, EXT], F32, tag="sp")
            softplus(spf[:], d_cur[:])
            lo = cpool.tile([128, W4, 2], F32)
            nc.vector.tensor_scalar(lo[:, :, 0], spf[:, HALO:EXT], -1.0,
                                    None, OP.mult)
            nc.vector.tensor_tensor(lo[:, :, 1], d_cur[:, HALO:EXT],
                                    spf[:, HALO:EXT], OP.subtract)
            out_view = outv.rearrange("(b k w) x -> (b k) w x", b=BL, k=16)
            nc.sync.dma_start(out_view, lo[:])

    nc.compile()
    _BUILD_CACHE[T] = nc
    return nc


# ---------------------------------------------------------------------------
# host-side prep + execution
# ---------------------------------------------------------------------------
def _prep_shared(inputs):
    sh = {}
    for l, (din_, kpad, wkey) in enumerate(((D_IN, 512, "w0"),
                                            (256, 384, "w1"))):
        for d, suf in (("f", ""), ("b", "r")):
            wih = _perm_gates(inputs[f"w_ih_l{l}{suf}"])       # [512, din]
            whh = _perm_gates(inputs[f"w_hh_l{l}{suf}"])       # [512, 128]
            bias = _perm_gates(
                np.asarray(inputs[f"b_ih_l{l}{suf}"])
                + np.asarray(inputs[f"b_hh_l{l}{suf}"]))       # [512]
            aug = np.zeros((kpad, 512), np.float32)
            aug[:din_] = np.asarray(wih, np.float32).T
            aug[din_] = bias
            sh[f"{wkey}{d}"] = _bf16(
                aug.reshape(kpad // 128, 128, 512).transpose(1, 0, 2))
            wT = np.asarray(whh, np.float32).T          # [128, 512]
            sh[f"whh{l}{d}"] = _bf16(
                np.concatenate([2.0 * wT, -wT], axis=1))  # h = 2*t2 - sig(o)

    g1, b1 = np.asarray(inputs["bn1_g"]), np.asarray(inputs["bn1_b"])
    m1, v1 = np.asarray(inputs["bn1_m"]), np.asarray(inputs["bn1_v"])
    s1 = g1 / np.sqrt(v1 + EPS)
    t1 = b1 - m1 * s1
    lin_w = np.asarray(inputs["lin_w"])
    LW = lin_w * s1[None, :]
    LB = np.asarray(inputs["lin_b"]) + lin_w @ t1
    g2, b2 = np.asarray(inputs["bn2_g"]), np.asarray(inputs["bn2_b"])
    m2, v2 = np.asarray(inputs["bn2_m"]), np.asarray(inputs["bn2_v"])
    s2 = g2 / np.sqrt(v2 + EPS)
    t2 = b2 - m2 * s2
    out_w, out_b = np.asarray(inputs["out_w"]), np.asarray(inputs["out_b"])
    W1, W2, W3 = out_w[:, 0:2], out_w[:, 2:4], out_w[:, 4:68]
    w3d = W3[1] - W3[0]
    K0 = (out_b[1] - out_b[0]) + t2 @ w3d
    w1d, w2d = W1[1] - W1[0], W2[1] - W2[0]
    alpha, beta = w1d[1], w1d[0] + w1d[1]
    gamma, delta = w2d[1], w2d[0] + w2d[1]

    sh["lwk"] = _bf16(LW.T.reshape(2, 128, 64).transpose(1, 0, 2))
    sh["lbv"] = np.asarray(LB, np.float32).reshape(64, 1)
    sh["w3s"] = _bf16((w3d * s2).reshape(64, 1))
    coefs = np.zeros((128, 8), np.float32)
    coefs[:, 0] = gamma
    coefs[:, 1] = -delta
    coefs[:, 2] = alpha
    coefs[:, 3] = -beta
    coefs[:, 4] = K0
    sh["coef"] = coefs
    cmsk = np.ones((128, 20), np.float32)
    cmsk[0::16, :] = 0.0          # sequence-start rows: zero halo
    sh["cmsk"] = cmsk
    sh["ident"] = _bf16(np.eye(128, dtype=np.float32))
    return sh


def _prep_core(x_core):
    # x_core: [BL, T, 400] -> aug kxn [128, 4, T*BL] bf16
    xt = np.zeros((512, T * BL), np.float32)
    xt[:D_IN] = np.asarray(x_core, np.float32).transpose(2, 1, 0).reshape(
        D_IN, T * BL)
    xt[D_IN] = 1.0
    return _bf16(xt.reshape(4, 128, T * BL).transpose(1, 0, 2))


def kernel(**inputs):
    nc = _build()
    sh = _prep_shared(inputs)
    x = np.asarray(inputs["x"], np.float32)
    in_maps = []
    for cidx in range(N_CORES):
        m = dict(sh)
        m["xk"] = _prep_core(x[cidx * BL:(cidx + 1) * BL])
        in_maps.append(m)
    res = run_bass_kernel_spmd(nc, in_maps, list(range(N_CORES)))
    outs = [np.asarray(res.results[i]["outv"], np.float32)
            for i in range(N_CORES)]
    return np.concatenate(outs, axis=0)


if __name__ == "__main__":
    import time
    t0 = time.time()
    print(f"building T={T}...")
    _build()
    print(f"built in {time.time() - t0:.1f}s")

